# revision 9
# baseline (speedup 1.0000x reference)
"""Trainium2 Bass kernel: 3D BFP activation quantization (shared-exponent blocks
of blk=16 contiguous channels along C), data-parallel over batch N across 8
NeuronCores.

kernel(activations[8,64,32,64,64] f32, mantissa=7, blk=16) -> same-shape f32.

Math (exact fp32/int32 bit arithmetic; matches the jnp reference bit-for-bit):
  per block b, spatial s:  M = max_i |x[blk*b+i, s]|
  E  = exponent field of M;  quantum = 2^(E-127-(mant-1));  scale = 1/quantum
  y   = x * scale                                  # exact (pow2), |y| < 2^mant
  t   = min(y + 1.5*2^23, 1.5*2^23 + lim)          # RNE round to int + high clip
  a   = relu(t - (1.5*2^23 - lim))                 # low clip; a = clipped + lim
  out = (a - lim) * quantum                        # exact

Bit manipulation (int32; Ei/Si/Qi are the exponent/scale/quantum bit patterns):
  Ei = max(Mbits & 0x7F800000, 0x0C800000)         # clamp E>=25 so zero blocks
                                                   #   get finite scale/quantum
  Si = 0x82000000 - Ei  (= ~Ei + 0x82000001)       # scale = 2^(133-E)
  Qi = Ei - ((mant-1)<<23)                         # quantum = 2^(E-133)

bf16 tail: a in [0, 2*lim] and (a-lim) in [-lim, lim] are small integers and
quantum is a power of two, so relu (ACT) -> bf16 and (a-lim) as a 4x-mode
tensor_scalar are exact in bf16; the final *quantum tensor_tensor reads
bf16 and writes fp32 straight into the store buffer (integer x pow2 in
fp32: exact, no output rounding). Output stays bit-identical to the fp32
reference while the DVE hot ops run at 2-4x.

Layout: partition = 128 spatial positions, free = (all 64 channels = 4
blocks, F=128 spatial); 16 DMAs of 4 MiB with 512B-contiguous DRAM chunks.
Measured on this HW, 512B-chunk APs move ~60% faster than 2KB-chunk ones
(258 us vs 408 us pure-copy per body), inverting the usual descriptor-size
guidance — so the DMA tiling stays in the baseline's AP shape. All DMAs on
nc.sync (HWDGE): the two HWDGE rings gave no extra throughput, and SWDGE
(cast-during-DMA) costs ~12 ns/descriptor of Q7 time on Pool (~25 us per
store here). Stage emission is software-pipelined (head/mid/tail/post) so
each engine's in-order queue stays dense; stores sit one stage after the
tail compute so SP never stalls waiting for data.
"""

import os
import sys

for _p in ("/opt/trn_rl_repo", "/root/.axon_site/_ro/trn_rl_repo"):
    if os.path.isdir(_p) and _p not in sys.path:
        sys.path.insert(0, _p)

import numpy as np

# ---- hardcoded problem geometry ----
N, C, D, H, W = 8, 64, 32, 64, 64
S = D * H * W                 # 131072 spatial per (n, c)
N_CORES = 8
PD = 128                      # SBUF partitions (spatial)

_BUILT = {}


def _build(mant: int, blk: int, F: int = 128, CT: int = 64, bufs: int = 5,
           abufs: int = 2, mbufs: int = 2, obufs: int = 0, round_df: int = 0,
           mult_engine: str = "gpsimd", relu_engine: str = "scalar",
           q16_engine: str = "scalar", copy_split: int = 0,
           smalls_engine: str = "vector",
           tail_mode: str = "ttdirect", upcast_engine: str = "vector",
           use_bf16: int = 1, store_engine: str = "sync", pair_loads: int = 0,
           C: int = C, S: int = S, mode: str = "full", repeat: int = 1):
    import concourse.bass as bass
    import concourse.bacc as bacc
    import concourse.mybir as mybir
    from concourse.tile import TileContext

    FP32 = mybir.dt.float32
    BF16 = mybir.dt.bfloat16
    I32 = mybir.dt.int32
    Alu = mybir.AluOpType

    NBt = CT // blk           # channel blocks per tile
    NCC = C // CT             # channel chunks
    NTS = S // (PD * F)       # spatial chunks
    assert S % (PD * F) == 0 and C % CT == 0 and CT % blk == 0

    LIM = float(2 ** mant - 1)
    MAGIC = 1.5 * 2.0 ** 23
    EXP_OFF = (mant - 1) << 23

    nc = bacc.Bacc("TRN2", target_bir_lowering=False)
    x_d = nc.dram_tensor("x", [C, S], FP32, kind="ExternalInput")
    o_d = nc.dram_tensor("o", [C, S], FP32, kind="ExternalOutput")

    xr = x_d[:].rearrange("(cc ct) (ts sh f) -> ts cc sh ct f",
                          cc=NCC, ct=CT, ts=NTS, sh=PD, f=F)
    orr = o_d[:].rearrange("(cc ct) (ts sh f) -> ts cc sh ct f",
                           cc=NCC, ct=CT, ts=NTS, sh=PD, f=F)
    if pair_loads:
        assert NCC == 1 and NTS % 2 == 0
        # one 2-wide load covers two consecutive ts chunks: [p, j, ct, f]
        xr2 = x_d[:].rearrange("(cc ct) (t2 j sh f) -> t2 cc sh j ct f",
                               cc=NCC, ct=CT, t2=NTS // 2, j=2, sh=PD, f=F)

    abufs = abufs or bufs
    mbufs = mbufs or bufs
    with TileContext(nc) as tc:
        with (
            tc.tile_pool(name="xp", bufs=bufs) as xp,
            tc.tile_pool(name="ap", bufs=abufs) as ap,
            tc.tile_pool(name="mp", bufs=mbufs) as mp,
            tc.tile_pool(name="qp", bufs=max(3, mbufs)) as qp,
            tc.tile_pool(name="op", bufs=max(obufs, 1)) as op_,
            tc.tile_pool(name="cp", bufs=1) as cp,
        ):
            relu_bias = cp.tile([PD, 1], FP32, tag="rbias")
            nc.vector.memset(relu_bias[:], -(MAGIC - LIM))
            tl = [(ts, cc) for ts in range(NTS) for cc in range(NCC)] * repeat

            # Software-pipelined emission: engines execute their streams
            # in order, so per-tile sequential emission exposes the
            # mult(Pool) and relu(ACT) latencies as DVE stalls. Skewing
            # the stages two deep keeps every engine's queue dense:
            #   iter i: head(i) [reduce+smalls+mult], mid(i-1)
            #           [round+relu], tail(i-2) [ts+tt+store]
            staged = {}

            def issue_load(i):
                ts_, cc_ = tl[i]
                if pair_loads:
                    if i % 2 == 0:
                        X2l = xp.tile([PD, 2 * CT, F], FP32, tag="x",
                                      name="X2l")
                        t2 = (i % (NTS * NCC)) // 2
                        nc.sync.dma_start(
                            X2l[:].rearrange("p (j c) f -> p j c f", j=2),
                            xr2[t2, cc_],
                        )
                        staged[i] = [X2l[:, :CT], None, None]
                        staged[i + 1] = [X2l[:, CT:], None, None]
                    return
                Xl = xp.tile([PD, CT, F], FP32, tag="x")
                nc.sync.dma_start(Xl[:], xr[ts_, cc_])
                staged[i] = [Xl, None, None]

            def head(i):
                X = staged[i][0]
                M = mp.tile([PD, NBt, F], FP32, tag="m")
                Q = mp.tile([PD, NBt, F], FP32, tag="q")
                Sc = mp.tile([PD, NBt, F], FP32, tag="s")
                Q16 = (qp.tile([PD, NBt, F], BF16, tag="q16", name="Q16")
                       if use_bf16 else None)
                # block abs-max over i: AP [p, b, f, i], innermost strided
                nc.vector.tensor_reduce(
                    M[:],
                    X[:].rearrange("p (b i) f -> p b f i", b=NBt, i=blk),
                    axis=mybir.AxisListType.X, op=Alu.max,
                    apply_absolute_value=True,
                )
                Mi = M[:].bitcast(I32)
                Qi = Q[:].bitcast(I32)
                Si = Sc[:].bitcast(I32)
                se = getattr(nc, smalls_engine)
                # op0/op1 must share an ALU class (bitwise vs arith), so 4 ops:
                # V = 0x7F800000 - Ebits   (pure bitwise complement trick)
                se.tensor_scalar(
                    Si, Mi, 0x807FFFFF - (1 << 32), -1,
                    op0=Alu.bitwise_or, op1=Alu.bitwise_xor,
                )
                # scale bits = min(V, 0x73000000) + 0x02800000
                se.tensor_scalar(
                    Si, Si, 0x73000000, 0x02800000,
                    op0=Alu.min, op1=Alu.add,
                )
                # Ebits
                se.tensor_scalar(
                    Qi, Mi, 0x7F800000, None, op0=Alu.bitwise_and
                )
                # quantum bits = max(Ebits, 0x0C800000) - ((mant-1)<<23)
                se.tensor_scalar(
                    Qi, Qi, 0x0C800000, EXP_OFF,
                    op0=Alu.max, op1=Alu.subtract,
                )
                if use_bf16:
                    # quantum as bf16 (exact: power of two)
                    if q16_engine == "gpsimd":
                        nc.gpsimd.tensor_copy(Q16[:], Q[:])
                    else:
                        nc.scalar.activation(
                            Q16[:], Q[:], mybir.ActivationFunctionType.Copy,
                            bias=0.0, scale=1.0,
                        )
                X4 = X[:].rearrange("p (b i) f -> p b i f", b=NBt, i=blk)
                Sb = Sc[:].unsqueeze(2).broadcast_to([PD, NBt, blk, F])
                # y = x * scale  (exact pow2 mult)
                getattr(nc, mult_engine).tensor_tensor(X4, X4, Sb, op=Alu.mult)
                if round_df:
                    # Pool's share of the round, chained right after mult on
                    # the same engine (no cross-engine stall).
                    nc.gpsimd.tensor_scalar(
                        X[:, :, F - round_df:], X[:, :, F - round_df:],
                        MAGIC, MAGIC + LIM, op0=Alu.add, op1=Alu.min,
                    )
                staged[i][2] = Q16 if use_bf16 else Q

            def mid(i):
                X = staged[i][0]
                if round_df < F:
                    # t = min(y + MAGIC, MAGIC + lim): RNE round + high clip
                    # (DVE's share; Pool did the last round_df columns)
                    nc.vector.tensor_scalar(
                        X[:, :, :F - round_df], X[:, :, :F - round_df],
                        MAGIC, MAGIC + LIM, op0=Alu.add, op1=Alu.min,
                    )
                X2 = X[:].rearrange("p c f -> p (c f)")
                # a = relu(t - (MAGIC - lim)): low clip; a in [0, 2*lim] ints
                if use_bf16:
                    A16 = ap.tile([PD, CT, F], BF16, tag="a")
                    A2 = A16[:].rearrange("p c f -> p (c f)")
                    if relu_engine == "vector":
                        # single-src 2-op tensor_scalar gets the 2x_2p mode
                        nc.vector.tensor_scalar(
                            A2, X2, MAGIC - LIM, 0.0,
                            op0=Alu.subtract, op1=Alu.max,
                        )
                    else:
                        nc.scalar.activation(
                            A2, X2, mybir.ActivationFunctionType.Relu,
                            bias=relu_bias[:], scale=1.0,
                        )
                    staged[i][1] = A16
                else:
                    if relu_engine == "scalar":
                        nc.scalar.activation(
                            X2, X2, mybir.ActivationFunctionType.Relu,
                            bias=relu_bias[:], scale=1.0,
                        )
                    else:
                        nc.vector.tensor_scalar(
                            X2, X2, MAGIC - LIM, 0.0,
                            op0=Alu.subtract, op1=Alu.max,
                        )
                    staged[i][1] = X

            def tail(i):
                X, A16, Qq = staged[i]
                if use_bf16 and tail_mode == "ttdirect":
                    # (a-lim) in bf16 (4x), then *quantum with fp32 output
                    # straight into X (1x, but saves the upcast op+hop)
                    A2 = A16[:].rearrange("p c f -> p (c f)")
                    nc.vector.tensor_scalar(
                        A2, A2, LIM, None, op0=Alu.subtract,
                    )
                    X4 = X[:].rearrange("p (b i) f -> p b i f", b=NBt, i=blk)
                    A4 = A16[:].rearrange("p (b i) f -> p b i f",
                                          b=NBt, i=blk)
                    for b in range(NBt):
                        Qb = Qq[:, b].unsqueeze(1).broadcast_to(
                            [PD, blk, F])
                        nc.vector.tensor_tensor(
                            X4[:, b], A4[:, b], Qb, op=Alu.mult)
                elif use_bf16 and tail_mode == "ttcopy":
                    # v2-proven tail: (a-lim) as 4x tensor_scalar, *quantum
                    # as 2x tensor_tensor (all bf16, exact), then bf16->f32
                    # upcast on ACT into X (free after relu); HWDGE store.
                    A2 = A16[:].rearrange("p c f -> p (c f)")
                    nc.vector.tensor_scalar(
                        A2, A2, LIM, None, op0=Alu.subtract,
                    )
                    for b in range(NBt):
                        A3 = A16[:, b * blk:(b + 1) * blk]
                        Qb = Qq[:, b].unsqueeze(1).broadcast_to(
                            [PD, blk, F])
                        nc.vector.tensor_tensor(A3, A3, Qb, op=Alu.mult)
                    X2 = X[:].rearrange("p c f -> p (c f)")
                    if upcast_engine == "vector":
                        # single-src copy w/ cast: 2x_2p on DVE
                        nc.vector.tensor_copy(X2, A2)
                    else:
                        nc.scalar.activation(
                            X2, A2, mybir.ActivationFunctionType.Copy,
                            bias=0.0, scale=1.0,
                        )
                elif use_bf16:
                    # out = (a - lim) * quantum: bf16 inputs, fp32 output
                    # written into X (free after relu) — exact, and the
                    # store stays on HWDGE (SWDGE descriptor generation is
                    # prohibitively slow for this access pattern).
                    X4 = X[:].rearrange("p (b i) f -> p b i f", b=NBt, i=blk)
                    A4 = A16[:].rearrange("p (b i) f -> p b i f",
                                          b=NBt, i=blk)
                    for b in range(NBt):
                        Qb = Qq[:, b].unsqueeze(1).broadcast_to(
                            [PD, blk, F])
                        nc.vector.scalar_tensor_tensor(
                            X4[:, b], A4[:, b], LIM, Qb,
                            op0=Alu.subtract, op1=Alu.mult,
                        )
                else:
                    X4 = X[:].rearrange("p (b i) f -> p b i f", b=NBt, i=blk)
                    for b in range(NBt):
                        Qb = Qq[:, b].unsqueeze(1).broadcast_to([PD, blk, F])
                        nc.vector.scalar_tensor_tensor(
                            X4[:, b], X4[:, b], LIM, Qb,
                            op0=Alu.subtract, op1=Alu.mult,
                        )

            def post(i):
                # Store stage, one iteration after tail: by the time the SP
                # sequencer reaches this dma_start, the data is long since
                # ready, so SP never stalls and never gates later loads.
                ts_, cc_ = tl[i]
                X = staged.pop(i)[0]
                getattr(nc, store_engine).dma_start(orr[ts_, cc_], X[:])

            n = len(tl)
            LB = bufs * 2 if pair_loads else bufs  # logical-tile lookahead
            for i in range(min(LB, n)):
                issue_load(i)
            if mode == "copy":
                st = nc.scalar if copy_split else nc.sync
                for i, (ts_, cc_) in enumerate(tl):
                    X = staged.pop(i)[0]
                    st.dma_start(orr[ts_, cc_], X[:])
                    if i + bufs < n:
                        issue_load(i + bufs)
            else:
                for i in range(n):
                    head(i)
                    if i >= 1:
                        mid(i - 1)
                    if i >= 2:
                        tail(i - 2)
                    if i >= 3:
                        post(i - 3)
                    # Emit the load that reuses the slot freed by the store
                    # just issued (post(i-3) frees X(i-3), which slot load
                    # (i-3+bufs) takes): keeps SP's in-order queue free of
                    # waits-on-later-entries (priority inversion).
                    j = i + LB - 3
                    if LB <= j < n:
                        issue_load(j)
                for j in (n - 1,):
                    mid(j)
                for j in (n - 2, n - 1):
                    if j >= 0:
                        tail(j)
                for j in (n - 3, n - 2, n - 1):
                    if j >= 0:
                        post(j)
    return nc


def _build_v2(mant: int, blk: int, CT: int = 32, F: int = 128, bufs: int = 6,
              abufs: int = 3, tbufs: int = 2, mbufs: int = 4,
              round_df: int = 0, mult_engine: str = "gpsimd",
              store_engine: str = "sync", load_engine: str = "sync",
              tail_mode: str = "stt",
              C: int = C, S: int = S, mode: str = "full", repeat: int = 1):
    """v2 pipeline: Scalar abs->bf16, DVE bf16 tree-max + int16 smalls,
    GpSimd broadcast scale-mult, DVE fp32 round, Scalar relu->bf16, DVE
    fused (a-lim)*q scalar_tensor_tensor with fp32 out.

    Engine loads per tile iter (CT=32, FD=4096/part), measured cost model:
      DVE   tree 2.2 + smalls 0.5 + scopy 0.2 + round 2.2 + stt 4.3 = 9.5 us
      Scalar abs 3.6 + relu 3.6 = 7.2 us
      GpSimd mult 9.8 us (2.5 cyc/elem two-input floor)
    x16 iters/body: DVE 151, GpSimd 157, Scalar 115 us -- all under the
    ~244 us/body DMA active floor, vs v1's DVE 310 us (strided reduce at
    2.5 cyc/elem + 4x broadcast-split tail TTs).
    """
    import concourse.bass as bass
    import concourse.bacc as bacc
    import concourse.mybir as mybir
    from concourse.tile import TileContext

    FP32 = mybir.dt.float32
    BF16 = mybir.dt.bfloat16
    I16 = mybir.dt.int16
    Alu = mybir.AluOpType
    Act = mybir.ActivationFunctionType

    NBt = CT // blk           # channel blocks per tile
    NCC = C // CT             # channel chunks
    NTS = S // (PD * F)       # spatial chunks
    assert S % (PD * F) == 0 and C % CT == 0 and CT % blk == 0 and blk == 16

    LIM = float(2 ** mant - 1)
    MAGIC = 1.5 * 2.0 ** 23
    EXP_OFF7 = (mant - 1) << 7

    nc = bacc.Bacc("TRN2", target_bir_lowering=False)
    x_d = nc.dram_tensor("x", [C, S], FP32, kind="ExternalInput")
    o_d = nc.dram_tensor("o", [C, S], FP32, kind="ExternalOutput")

    xr = x_d[:].rearrange("(cc ct) (ts sh f) -> ts cc sh ct f",
                          cc=NCC, ct=CT, ts=NTS, sh=PD, f=F)
    orr = o_d[:].rearrange("(cc ct) (ts sh f) -> ts cc sh ct f",
                           cc=NCC, ct=CT, ts=NTS, sh=PD, f=F)

    with TileContext(nc) as tc:
        with (
            tc.tile_pool(name="xp", bufs=bufs) as xp,
            tc.tile_pool(name="ap", bufs=abufs) as ap,      # abs bf16
            tc.tile_pool(name="rp", bufs=abufs) as rp,      # relu bf16
            tc.tile_pool(name="tp", bufs=tbufs) as tp,      # tree scratch
            tc.tile_pool(name="mp", bufs=mbufs) as mp,      # block smalls
            tc.tile_pool(name="cp", bufs=1) as cp,
        ):
            relu_bias = cp.tile([PD, 1], FP32, tag="rbias")
            nc.vector.memset(relu_bias[:], -(MAGIC - LIM))
            tl = [(ts, cc) for ts in range(NTS) for cc in range(NCC)] * repeat
            staged = {}

            def issue_load(i):
                ts_, cc_ = tl[i]
                Xl = xp.tile([PD, CT, F], FP32, tag="x")
                getattr(nc, load_engine).dma_start(Xl[:], xr[ts_, cc_])
                staged[i] = [Xl, None, None]

            def h_abs(i):
                X = staged[i][0]
                AB = ap.tile([PD, CT, F], BF16, tag="ab")
                nc.scalar.activation(
                    AB[:].rearrange("p c f -> p (c f)"),
                    X[:].rearrange("p c f -> p (c f)"),
                    Act.Abs, bias=0.0, scale=1.0,
                )
                staged[i].append(AB)

            def h_tree(i):
                X, _, _, AB = staged[i]
                A4 = AB[:].rearrange("p (b i) f -> p b i f", b=NBt, i=blk)
                T1 = tp.tile([PD, NBt, 8, F], BF16, tag="t1")
                T2 = tp.tile([PD, NBt, 4, F], BF16, tag="t2")
                T3 = tp.tile([PD, NBt, 2, F], BF16, tag="t3")
                M16 = mp.tile([PD, NBt, F], BF16, tag="m16", name="M16")
                S16 = mp.tile([PD, NBt, F], BF16, tag="s16", name="S16")
                Q16 = mp.tile([PD, NBt, F], BF16, tag="q16", name="Q16")
                S32 = mp.tile([PD, NBt, F], FP32, tag="s32", name="S32")
                v = nc.vector

                def half(t, k):
                    # [p, b, 2k, f] -> lo/hi halves as 3D [p, b, k*f]
                    t4 = t if len(t.shape) == 4 else t
                    lo = t4[:, :, 0:k].rearrange("p b i f -> p b (i f)")
                    hi = t4[:, :, k:2 * k].rearrange("p b i f -> p b (i f)")
                    return lo, hi

                lo, hi = half(A4, 8)
                v.tensor_tensor(T1[:].rearrange("p b i f -> p b (i f)"),
                                lo, hi, op=Alu.max)
                lo, hi = half(T1[:], 4)
                v.tensor_tensor(T2[:].rearrange("p b i f -> p b (i f)"),
                                lo, hi, op=Alu.max)
                lo, hi = half(T2[:], 2)
                v.tensor_tensor(T3[:].rearrange("p b i f -> p b (i f)"),
                                lo, hi, op=Alu.max)
                lo, hi = half(T3[:], 1)
                v.tensor_tensor(M16[:], lo, hi, op=Alu.max)
                Mi = M16[:].bitcast(I16)
                Si = S16[:].bitcast(I16)
                Qi = Q16[:].bitcast(I16)
                # V = 0x7F80 - Ebits via complement trick (sign bit is 0)
                v.tensor_scalar(Si, Mi, 0x807F - (1 << 16), -1,
                                op0=Alu.bitwise_or, op1=Alu.bitwise_xor)
                # scale bits = min(V, 0x7300) + 0x0280  -> 2^(133-max(E,25))
                v.tensor_scalar(Si, Si, 0x7300, 0x0280,
                                op0=Alu.min, op1=Alu.add)
                v.tensor_scalar(Qi, Mi, 0x7F80, None, op0=Alu.bitwise_and)
                # quantum bits = max(Ebits, 0x0C80) - ((mant-1)<<7)
                v.tensor_scalar(Qi, Qi, 0x0C80, EXP_OFF7,
                                op0=Alu.max, op1=Alu.subtract)
                v.tensor_copy(S32[:], S16[:])
                staged[i][1] = Q16
                staged[i][2] = S32

            def h_mult(i):
                X, _, S32 = staged[i][:3]
                X4 = X[:].rearrange("p (b i) f -> p b i f", b=NBt, i=blk)
                Sb = S32[:].unsqueeze(2).broadcast_to([PD, NBt, blk, F])
                getattr(nc, mult_engine).tensor_tensor(X4, X4, Sb,
                                                       op=Alu.mult)
                if round_df:
                    nc.gpsimd.tensor_scalar(
                        X[:, :, F - round_df:], X[:, :, F - round_df:],
                        MAGIC, MAGIC + LIM, op0=Alu.add, op1=Alu.min,
                    )

            def mid(i):
                X = staged[i][0]
                if round_df < F:
                    nc.vector.tensor_scalar(
                        X[:, :, :F - round_df], X[:, :, :F - round_df],
                        MAGIC, MAGIC + LIM, op0=Alu.add, op1=Alu.min,
                    )
                A16 = rp.tile([PD, CT, F], BF16, tag="a16")
                nc.scalar.activation(
                    A16[:].rearrange("p c f -> p (c f)"),
                    X[:].rearrange("p c f -> p (c f)"),
                    Act.Relu, bias=relu_bias[:], scale=1.0,
                )
                staged[i].append(A16)

            def tail(i):
                X, Q16 = staged[i][0], staged[i][1]
                A16 = staged[i][4]
                X4 = X[:].rearrange("p (b i) f -> p b (i f)", b=NBt, i=blk)
                A4 = A16[:].rearrange("p (b i) f -> p b (i f)", b=NBt, i=blk)
                if tail_mode == "stt":
                    for b in range(NBt):
                        Qb = Q16[:, b].unsqueeze(1).broadcast_to(
                            [PD, blk, F])
                        nc.vector.scalar_tensor_tensor(
                            X4[:, b], A4[:, b], LIM, Qb,
                            op0=Alu.subtract, op1=Alu.mult,
                        )
                else:  # ttdirect: bf16 4x sub, then per-block bc TT
                    A2 = A16[:].rearrange("p c f -> p (c f)")
                    nc.vector.tensor_scalar(A2, A2, LIM, None,
                                            op0=Alu.subtract)
                    for b in range(NBt):
                        Qb = Q16[:, b].unsqueeze(1).broadcast_to(
                            [PD, blk, F])
                        nc.vector.tensor_tensor(X4[:, b], A4[:, b], Qb,
                                                op=Alu.mult)

            def post(i):
                ts_, cc_ = tl[i]
                X = staged.pop(i)[0]
                getattr(nc, store_engine).dma_start(orr[ts_, cc_], X[:])

            n = len(tl)
            for i in range(min(bufs, n)):
                issue_load(i)
            if mode == "copy":
                for i, (ts_, cc_) in enumerate(tl):
                    X = staged.pop(i)[0]
                    nc.sync.dma_start(orr[ts_, cc_], X[:])
                    if i + bufs < n:
                        issue_load(i + bufs)
            else:
                # skew: abs(i) | tree+mult(i-1) | round+relu(i-2) |
                #       stt(i-3) | store(i-4)
                for i in range(n):
                    h_abs(i)
                    if i >= 1:
                        h_tree(i - 1)
                        h_mult(i - 1)
                    if i >= 2:
                        mid(i - 2)
                    if i >= 3:
                        tail(i - 3)
                    if i >= 4:
                        post(i - 4)
                    j = i + bufs - 4
                    if bufs <= j < n:
                        issue_load(j)
                for j in (n - 1,):
                    h_tree(j)
                    h_mult(j)
                for j in (n - 2, n - 1):
                    if j >= 0:
                        mid(j)
                for j in (n - 3, n - 2, n - 1):
                    if j >= 0:
                        tail(j)
                for j in (n - 4, n - 3, n - 2, n - 1):
                    if j >= 0:
                        post(j)
    return nc


def _get_nc(mant: int, blk: int, arch: str = "v2", **kw):
    key = (mant, blk, arch, tuple(sorted(kw.items())))
    if key not in _BUILT:
        build = {"v1": _build, "v2": _build_v2}[arch]
        nc = build(mant, blk, **kw)
        if not nc.is_finalized():
            nc.finalize()
        _BUILT[key] = nc
    return _BUILT[key]


def kernel(activations, mantissa=7, blk=16, **_ignored):
    from concourse.bass_utils import run_bass_kernel_spmd

    mant = int(np.asarray(mantissa))
    blk = int(np.asarray(blk))
    x = np.asarray(activations, dtype=np.float32)
    assert x.shape == (N, C, D, H, W), x.shape
    assert blk == 16 and C % blk == 0

    nc = _get_nc(mant, blk)
    xf = x.reshape(N, C, S)
    in_maps = [{"x": np.ascontiguousarray(xf[n])} for n in range(N_CORES)]
    res = run_bass_kernel_spmd(nc, in_maps, list(range(N_CORES)))
    outs = [np.asarray(r["o"], dtype=np.float32) for r in res.results]
    return np.stack(outs, axis=0).reshape(N, C, D, H, W)



# revision 10
# speedup vs baseline: 2.3996x; 2.3996x over previous
"""Trainium2 Bass kernel: 3D BFP activation quantization (shared-exponent blocks
of blk=16 contiguous channels along C), data-parallel over batch N across 8
NeuronCores.

kernel(activations[8,64,32,64,64] f32, mantissa=7, blk=16) -> same-shape f32.

Math (exact fp32/int32 bit arithmetic; matches the jnp reference bit-for-bit):
  per block b, spatial s:  M = max_i |x[blk*b+i, s]|
  E  = exponent field of M;  quantum = 2^(E-127-(mant-1));  scale = 1/quantum
  y   = x * scale                                  # exact (pow2), |y| < 2^mant
  t   = min(y + 1.5*2^23, 1.5*2^23 + lim)          # RNE round to int + high clip
  a   = relu(t - (1.5*2^23 - lim))                 # low clip; a = clipped + lim
  out = (a - lim) * quantum                        # exact

Bit manipulation (int32; Ei/Si/Qi are the exponent/scale/quantum bit patterns):
  Ei = max(Mbits & 0x7F800000, 0x0C800000)         # clamp E>=25 so zero blocks
                                                   #   get finite scale/quantum
  Si = 0x82000000 - Ei  (= ~Ei + 0x82000001)       # scale = 2^(133-E)
  Qi = Ei - ((mant-1)<<23)                         # quantum = 2^(E-133)

bf16 tail: a in [0, 2*lim] and (a-lim) in [-lim, lim] are small integers and
quantum is a power of two, so relu (ACT) -> bf16 and (a-lim) as a 4x-mode
tensor_scalar are exact in bf16; the final *quantum tensor_tensor reads
bf16 and writes fp32 straight into the store buffer (integer x pow2 in
fp32: exact, no output rounding). Output stays bit-identical to the fp32
reference while the DVE hot ops run at 2-4x.

Layout: partition = 128 spatial positions, free = (all 64 channels = 4
blocks, F=128 spatial); 16 DMAs of 4 MiB with 512B-contiguous DRAM chunks.
Measured on this HW, 512B-chunk APs move ~60% faster than 2KB-chunk ones
(258 us vs 408 us pure-copy per body), inverting the usual descriptor-size
guidance — so the DMA tiling stays in the baseline's AP shape. All DMAs on
nc.sync (HWDGE): the two HWDGE rings gave no extra throughput, and SWDGE
(cast-during-DMA) costs ~12 ns/descriptor of Q7 time on Pool (~25 us per
store here). Stage emission is software-pipelined (head/mid/tail/post) so
each engine's in-order queue stays dense; stores sit one stage after the
tail compute so SP never stalls waiting for data.
"""

import os
import sys

for _p in ("/opt/trn_rl_repo", "/root/.axon_site/_ro/trn_rl_repo"):
    if os.path.isdir(_p) and _p not in sys.path:
        sys.path.insert(0, _p)

import numpy as np

# ---- hardcoded problem geometry ----
N, C, D, H, W = 8, 64, 32, 64, 64
S = D * H * W                 # 131072 spatial per (n, c)
N_CORES = 8
PD = 128                      # SBUF partitions (spatial)

_BUILT = {}


def _build(mant: int, blk: int, F: int = 128, CT: int = 64, bufs: int = 5,
           abufs: int = 2, mbufs: int = 2, obufs: int = 0, round_df: int = 0,
           mult_engine: str = "gpsimd", relu_engine: str = "scalar",
           q16_engine: str = "scalar", copy_split: int = 0,
           smalls_engine: str = "vector",
           tail_mode: str = "ttdirect", upcast_engine: str = "vector",
           use_bf16: int = 1, store_engine: str = "sync", pair_loads: int = 0,
           C: int = C, S: int = S, mode: str = "full", repeat: int = 1):
    import concourse.bass as bass
    import concourse.bacc as bacc
    import concourse.mybir as mybir
    from concourse.tile import TileContext

    FP32 = mybir.dt.float32
    BF16 = mybir.dt.bfloat16
    I32 = mybir.dt.int32
    Alu = mybir.AluOpType

    NBt = CT // blk           # channel blocks per tile
    NCC = C // CT             # channel chunks
    NTS = S // (PD * F)       # spatial chunks
    assert S % (PD * F) == 0 and C % CT == 0 and CT % blk == 0

    LIM = float(2 ** mant - 1)
    MAGIC = 1.5 * 2.0 ** 23
    EXP_OFF = (mant - 1) << 23

    nc = bacc.Bacc("TRN2", target_bir_lowering=False)
    x_d = nc.dram_tensor("x", [C, S], FP32, kind="ExternalInput")
    o_d = nc.dram_tensor("o", [C, S], FP32, kind="ExternalOutput")

    xr = x_d[:].rearrange("(cc ct) (ts sh f) -> ts cc sh ct f",
                          cc=NCC, ct=CT, ts=NTS, sh=PD, f=F)
    orr = o_d[:].rearrange("(cc ct) (ts sh f) -> ts cc sh ct f",
                           cc=NCC, ct=CT, ts=NTS, sh=PD, f=F)
    if pair_loads:
        assert NCC == 1 and NTS % 2 == 0
        # one 2-wide load covers two consecutive ts chunks: [p, j, ct, f]
        xr2 = x_d[:].rearrange("(cc ct) (t2 j sh f) -> t2 cc sh j ct f",
                               cc=NCC, ct=CT, t2=NTS // 2, j=2, sh=PD, f=F)

    abufs = abufs or bufs
    mbufs = mbufs or bufs
    with TileContext(nc) as tc:
        with (
            tc.tile_pool(name="xp", bufs=bufs) as xp,
            tc.tile_pool(name="ap", bufs=abufs) as ap,
            tc.tile_pool(name="mp", bufs=mbufs) as mp,
            tc.tile_pool(name="qp", bufs=max(3, mbufs)) as qp,
            tc.tile_pool(name="op", bufs=max(obufs, 1)) as op_,
            tc.tile_pool(name="cp", bufs=1) as cp,
        ):
            relu_bias = cp.tile([PD, 1], FP32, tag="rbias")
            nc.vector.memset(relu_bias[:], -(MAGIC - LIM))
            tl = [(ts, cc) for ts in range(NTS) for cc in range(NCC)] * repeat

            # Software-pipelined emission: engines execute their streams
            # in order, so per-tile sequential emission exposes the
            # mult(Pool) and relu(ACT) latencies as DVE stalls. Skewing
            # the stages two deep keeps every engine's queue dense:
            #   iter i: head(i) [reduce+smalls+mult], mid(i-1)
            #           [round+relu], tail(i-2) [ts+tt+store]
            staged = {}

            def issue_load(i):
                ts_, cc_ = tl[i]
                if pair_loads:
                    if i % 2 == 0:
                        X2l = xp.tile([PD, 2 * CT, F], FP32, tag="x",
                                      name="X2l")
                        t2 = (i % (NTS * NCC)) // 2
                        nc.sync.dma_start(
                            X2l[:].rearrange("p (j c) f -> p j c f", j=2),
                            xr2[t2, cc_],
                        )
                        staged[i] = [X2l[:, :CT], None, None]
                        staged[i + 1] = [X2l[:, CT:], None, None]
                    return
                Xl = xp.tile([PD, CT, F], FP32, tag="x")
                nc.sync.dma_start(Xl[:], xr[ts_, cc_])
                staged[i] = [Xl, None, None]

            def head(i):
                X = staged[i][0]
                M = mp.tile([PD, NBt, F], FP32, tag="m")
                Q = mp.tile([PD, NBt, F], FP32, tag="q")
                Sc = mp.tile([PD, NBt, F], FP32, tag="s")
                Q16 = (qp.tile([PD, NBt, F], BF16, tag="q16", name="Q16")
                       if use_bf16 else None)
                # block abs-max over i: AP [p, b, f, i], innermost strided
                nc.vector.tensor_reduce(
                    M[:],
                    X[:].rearrange("p (b i) f -> p b f i", b=NBt, i=blk),
                    axis=mybir.AxisListType.X, op=Alu.max,
                    apply_absolute_value=True,
                )
                Mi = M[:].bitcast(I32)
                Qi = Q[:].bitcast(I32)
                Si = Sc[:].bitcast(I32)
                se = getattr(nc, smalls_engine)
                # op0/op1 must share an ALU class (bitwise vs arith), so 4 ops:
                # V = 0x7F800000 - Ebits   (pure bitwise complement trick)
                se.tensor_scalar(
                    Si, Mi, 0x807FFFFF - (1 << 32), -1,
                    op0=Alu.bitwise_or, op1=Alu.bitwise_xor,
                )
                # scale bits = min(V, 0x73000000) + 0x02800000
                se.tensor_scalar(
                    Si, Si, 0x73000000, 0x02800000,
                    op0=Alu.min, op1=Alu.add,
                )
                # Ebits
                se.tensor_scalar(
                    Qi, Mi, 0x7F800000, None, op0=Alu.bitwise_and
                )
                # quantum bits = max(Ebits, 0x0C800000) - ((mant-1)<<23)
                se.tensor_scalar(
                    Qi, Qi, 0x0C800000, EXP_OFF,
                    op0=Alu.max, op1=Alu.subtract,
                )
                if use_bf16:
                    # quantum as bf16 (exact: power of two)
                    if q16_engine == "gpsimd":
                        nc.gpsimd.tensor_copy(Q16[:], Q[:])
                    else:
                        nc.scalar.activation(
                            Q16[:], Q[:], mybir.ActivationFunctionType.Copy,
                            bias=0.0, scale=1.0,
                        )
                X4 = X[:].rearrange("p (b i) f -> p b i f", b=NBt, i=blk)
                Sb = Sc[:].unsqueeze(2).broadcast_to([PD, NBt, blk, F])
                # y = x * scale  (exact pow2 mult)
                getattr(nc, mult_engine).tensor_tensor(X4, X4, Sb, op=Alu.mult)
                if round_df:
                    # Pool's share of the round, chained right after mult on
                    # the same engine (no cross-engine stall).
                    nc.gpsimd.tensor_scalar(
                        X[:, :, F - round_df:], X[:, :, F - round_df:],
                        MAGIC, MAGIC + LIM, op0=Alu.add, op1=Alu.min,
                    )
                staged[i][2] = Q16 if use_bf16 else Q

            def mid(i):
                X = staged[i][0]
                if round_df < F:
                    # t = min(y + MAGIC, MAGIC + lim): RNE round + high clip
                    # (DVE's share; Pool did the last round_df columns)
                    nc.vector.tensor_scalar(
                        X[:, :, :F - round_df], X[:, :, :F - round_df],
                        MAGIC, MAGIC + LIM, op0=Alu.add, op1=Alu.min,
                    )
                X2 = X[:].rearrange("p c f -> p (c f)")
                # a = relu(t - (MAGIC - lim)): low clip; a in [0, 2*lim] ints
                if use_bf16:
                    A16 = ap.tile([PD, CT, F], BF16, tag="a")
                    A2 = A16[:].rearrange("p c f -> p (c f)")
                    if relu_engine == "vector":
                        # single-src 2-op tensor_scalar gets the 2x_2p mode
                        nc.vector.tensor_scalar(
                            A2, X2, MAGIC - LIM, 0.0,
                            op0=Alu.subtract, op1=Alu.max,
                        )
                    else:
                        nc.scalar.activation(
                            A2, X2, mybir.ActivationFunctionType.Relu,
                            bias=relu_bias[:], scale=1.0,
                        )
                    staged[i][1] = A16
                else:
                    if relu_engine == "scalar":
                        nc.scalar.activation(
                            X2, X2, mybir.ActivationFunctionType.Relu,
                            bias=relu_bias[:], scale=1.0,
                        )
                    else:
                        nc.vector.tensor_scalar(
                            X2, X2, MAGIC - LIM, 0.0,
                            op0=Alu.subtract, op1=Alu.max,
                        )
                    staged[i][1] = X

            def tail(i):
                X, A16, Qq = staged[i]
                if use_bf16 and tail_mode == "ttdirect":
                    # (a-lim) in bf16 (4x), then *quantum with fp32 output
                    # straight into X (1x, but saves the upcast op+hop)
                    A2 = A16[:].rearrange("p c f -> p (c f)")
                    nc.vector.tensor_scalar(
                        A2, A2, LIM, None, op0=Alu.subtract,
                    )
                    X4 = X[:].rearrange("p (b i) f -> p b i f", b=NBt, i=blk)
                    A4 = A16[:].rearrange("p (b i) f -> p b i f",
                                          b=NBt, i=blk)
                    for b in range(NBt):
                        Qb = Qq[:, b].unsqueeze(1).broadcast_to(
                            [PD, blk, F])
                        nc.vector.tensor_tensor(
                            X4[:, b], A4[:, b], Qb, op=Alu.mult)
                elif use_bf16 and tail_mode == "ttcopy":
                    # v2-proven tail: (a-lim) as 4x tensor_scalar, *quantum
                    # as 2x tensor_tensor (all bf16, exact), then bf16->f32
                    # upcast on ACT into X (free after relu); HWDGE store.
                    A2 = A16[:].rearrange("p c f -> p (c f)")
                    nc.vector.tensor_scalar(
                        A2, A2, LIM, None, op0=Alu.subtract,
                    )
                    for b in range(NBt):
                        A3 = A16[:, b * blk:(b + 1) * blk]
                        Qb = Qq[:, b].unsqueeze(1).broadcast_to(
                            [PD, blk, F])
                        nc.vector.tensor_tensor(A3, A3, Qb, op=Alu.mult)
                    X2 = X[:].rearrange("p c f -> p (c f)")
                    if upcast_engine == "vector":
                        # single-src copy w/ cast: 2x_2p on DVE
                        nc.vector.tensor_copy(X2, A2)
                    else:
                        nc.scalar.activation(
                            X2, A2, mybir.ActivationFunctionType.Copy,
                            bias=0.0, scale=1.0,
                        )
                elif use_bf16:
                    # out = (a - lim) * quantum: bf16 inputs, fp32 output
                    # written into X (free after relu) — exact, and the
                    # store stays on HWDGE (SWDGE descriptor generation is
                    # prohibitively slow for this access pattern).
                    X4 = X[:].rearrange("p (b i) f -> p b i f", b=NBt, i=blk)
                    A4 = A16[:].rearrange("p (b i) f -> p b i f",
                                          b=NBt, i=blk)
                    for b in range(NBt):
                        Qb = Qq[:, b].unsqueeze(1).broadcast_to(
                            [PD, blk, F])
                        nc.vector.scalar_tensor_tensor(
                            X4[:, b], A4[:, b], LIM, Qb,
                            op0=Alu.subtract, op1=Alu.mult,
                        )
                else:
                    X4 = X[:].rearrange("p (b i) f -> p b i f", b=NBt, i=blk)
                    for b in range(NBt):
                        Qb = Qq[:, b].unsqueeze(1).broadcast_to([PD, blk, F])
                        nc.vector.scalar_tensor_tensor(
                            X4[:, b], X4[:, b], LIM, Qb,
                            op0=Alu.subtract, op1=Alu.mult,
                        )

            def post(i):
                # Store stage, one iteration after tail: by the time the SP
                # sequencer reaches this dma_start, the data is long since
                # ready, so SP never stalls and never gates later loads.
                ts_, cc_ = tl[i]
                X = staged.pop(i)[0]
                getattr(nc, store_engine).dma_start(orr[ts_, cc_], X[:])

            n = len(tl)
            LB = bufs * 2 if pair_loads else bufs  # logical-tile lookahead
            for i in range(min(LB, n)):
                issue_load(i)
            if mode == "copy":
                st = nc.scalar if copy_split else nc.sync
                for i, (ts_, cc_) in enumerate(tl):
                    X = staged.pop(i)[0]
                    st.dma_start(orr[ts_, cc_], X[:])
                    if i + bufs < n:
                        issue_load(i + bufs)
            else:
                for i in range(n):
                    head(i)
                    if i >= 1:
                        mid(i - 1)
                    if i >= 2:
                        tail(i - 2)
                    if i >= 3:
                        post(i - 3)
                    # Emit the load that reuses the slot freed by the store
                    # just issued (post(i-3) frees X(i-3), which slot load
                    # (i-3+bufs) takes): keeps SP's in-order queue free of
                    # waits-on-later-entries (priority inversion).
                    j = i + LB - 3
                    if LB <= j < n:
                        issue_load(j)
                for j in (n - 1,):
                    mid(j)
                for j in (n - 2, n - 1):
                    if j >= 0:
                        tail(j)
                for j in (n - 3, n - 2, n - 1):
                    if j >= 0:
                        post(j)
    return nc


def _build_v2(mant: int, blk: int, CT: int = 32, F: int = 128, bufs: int = 6,
              abufs: int = 3, tbufs: int = 2, mbufs: int = 4,
              round_df: int = 0, mult_engine: str = "gpsimd",
              store_engine: str = "sync", load_engine: str = "sync",
              tail_mode: str = "ttdirect",
              C: int = C, S: int = S, mode: str = "full", repeat: int = 1):
    """v2 pipeline: Scalar abs->bf16, DVE bf16 tree-max + int16 smalls,
    GpSimd broadcast scale-mult, DVE fp32 round, Scalar relu->bf16, DVE
    fused (a-lim)*q scalar_tensor_tensor with fp32 out.

    Engine loads per tile iter (CT=32, FD=4096/part), measured cost model:
      DVE   tree 2.2 + smalls 0.5 + scopy 0.2 + round 2.2 + stt 4.3 = 9.5 us
      Scalar abs 3.6 + relu 3.6 = 7.2 us
      GpSimd mult 9.8 us (2.5 cyc/elem two-input floor)
    x16 iters/body: DVE 151, GpSimd 157, Scalar 115 us -- all under the
    ~244 us/body DMA active floor, vs v1's DVE 310 us (strided reduce at
    2.5 cyc/elem + 4x broadcast-split tail TTs).
    """
    import concourse.bass as bass
    import concourse.bacc as bacc
    import concourse.mybir as mybir
    from concourse.tile import TileContext

    FP32 = mybir.dt.float32
    BF16 = mybir.dt.bfloat16
    I16 = mybir.dt.int16
    Alu = mybir.AluOpType
    Act = mybir.ActivationFunctionType

    NBt = CT // blk           # channel blocks per tile
    NCC = C // CT             # channel chunks
    NTS = S // (PD * F)       # spatial chunks
    assert S % (PD * F) == 0 and C % CT == 0 and CT % blk == 0 and blk == 16

    LIM = float(2 ** mant - 1)
    MAGIC = 1.5 * 2.0 ** 23
    EXP_OFF7 = (mant - 1) << 7

    nc = bacc.Bacc("TRN2", target_bir_lowering=False)
    x_d = nc.dram_tensor("x", [C, S], FP32, kind="ExternalInput")
    o_d = nc.dram_tensor("o", [C, S], FP32, kind="ExternalOutput")

    xr = x_d[:].rearrange("(cc ct) (ts sh f) -> ts cc sh ct f",
                          cc=NCC, ct=CT, ts=NTS, sh=PD, f=F)
    orr = o_d[:].rearrange("(cc ct) (ts sh f) -> ts cc sh ct f",
                           cc=NCC, ct=CT, ts=NTS, sh=PD, f=F)

    with TileContext(nc) as tc:
        with (
            tc.tile_pool(name="xp", bufs=bufs) as xp,
            tc.tile_pool(name="ap", bufs=abufs) as ap,      # abs bf16
            tc.tile_pool(name="rp", bufs=abufs) as rp,      # relu bf16
            tc.tile_pool(name="tp", bufs=tbufs) as tp,      # tree scratch
            tc.tile_pool(name="mp", bufs=mbufs) as mp,      # block smalls
            tc.tile_pool(name="cp", bufs=1) as cp,
        ):
            relu_bias = cp.tile([PD, 1], FP32, tag="rbias")
            nc.vector.memset(relu_bias[:], -(MAGIC - LIM))
            tl = [(ts, cc) for ts in range(NTS) for cc in range(NCC)] * repeat
            staged = {}

            def issue_load(i):
                ts_, cc_ = tl[i]
                Xl = xp.tile([PD, CT, F], FP32, tag="x")
                getattr(nc, load_engine).dma_start(Xl[:], xr[ts_, cc_])
                staged[i] = [Xl, None, None]

            def h_abs(i):
                X = staged[i][0]
                AB = ap.tile([PD, CT, F], BF16, tag="ab")
                nc.scalar.activation(
                    AB[:].rearrange("p c f -> p (c f)"),
                    X[:].rearrange("p c f -> p (c f)"),
                    Act.Abs, bias=0.0, scale=1.0,
                )
                staged[i].append(AB)

            def h_tree(i):
                X, _, _, AB = staged[i]
                A4 = AB[:].rearrange("p (b i) f -> p b i f", b=NBt, i=blk)
                T1 = tp.tile([PD, NBt, 8, F], BF16, tag="t1")
                T2 = tp.tile([PD, NBt, 4, F], BF16, tag="t2")
                T3 = tp.tile([PD, NBt, 2, F], BF16, tag="t3")
                M16 = mp.tile([PD, NBt, F], BF16, tag="m16", name="M16")
                S16 = mp.tile([PD, NBt, F], BF16, tag="s16", name="S16")
                Q16 = mp.tile([PD, NBt, F], BF16, tag="q16", name="Q16")
                S32 = mp.tile([PD, NBt, F], FP32, tag="s32", name="S32")
                v = nc.vector

                def half(t, k):
                    # [p, b, 2k, f] -> lo/hi halves as 3D [p, b, k*f]
                    t4 = t if len(t.shape) == 4 else t
                    lo = t4[:, :, 0:k].rearrange("p b i f -> p b (i f)")
                    hi = t4[:, :, k:2 * k].rearrange("p b i f -> p b (i f)")
                    return lo, hi

                lo, hi = half(A4, 8)
                v.tensor_tensor(T1[:].rearrange("p b i f -> p b (i f)"),
                                lo, hi, op=Alu.max)
                lo, hi = half(T1[:], 4)
                v.tensor_tensor(T2[:].rearrange("p b i f -> p b (i f)"),
                                lo, hi, op=Alu.max)
                lo, hi = half(T2[:], 2)
                v.tensor_tensor(T3[:].rearrange("p b i f -> p b (i f)"),
                                lo, hi, op=Alu.max)
                lo, hi = half(T3[:], 1)
                v.tensor_tensor(M16[:], lo, hi, op=Alu.max)
                Mi = M16[:].bitcast(I16)
                Si = S16[:].bitcast(I16)
                Qi = Q16[:].bitcast(I16)
                # V = 0x7F80 - Ebits via complement trick (sign bit is 0)
                v.tensor_scalar(Si, Mi, 0x807F - (1 << 16), -1,
                                op0=Alu.bitwise_or, op1=Alu.bitwise_xor)
                # scale bits = min(V, 0x7300) + 0x0280  -> 2^(133-max(E,25))
                v.tensor_scalar(Si, Si, 0x7300, 0x0280,
                                op0=Alu.min, op1=Alu.add)
                v.tensor_scalar(Qi, Mi, 0x7F80, None, op0=Alu.bitwise_and)
                # quantum bits = max(Ebits, 0x0C80) - ((mant-1)<<7)
                v.tensor_scalar(Qi, Qi, 0x0C80, EXP_OFF7,
                                op0=Alu.max, op1=Alu.subtract)
                v.tensor_copy(S32[:], S16[:])
                staged[i][1] = Q16
                staged[i][2] = S32

            def h_mult(i):
                X, _, S32 = staged[i][:3]
                X4 = X[:].rearrange("p (b i) f -> p b i f", b=NBt, i=blk)
                Sb = S32[:].unsqueeze(2).broadcast_to([PD, NBt, blk, F])
                getattr(nc, mult_engine).tensor_tensor(X4, X4, Sb,
                                                       op=Alu.mult)
                if round_df:
                    nc.gpsimd.tensor_scalar(
                        X[:, :, F - round_df:], X[:, :, F - round_df:],
                        MAGIC, MAGIC + LIM, op0=Alu.add, op1=Alu.min,
                    )

            def mid(i):
                X = staged[i][0]
                if round_df < F:
                    nc.vector.tensor_scalar(
                        X[:, :, :F - round_df], X[:, :, :F - round_df],
                        MAGIC, MAGIC + LIM, op0=Alu.add, op1=Alu.min,
                    )
                A16 = rp.tile([PD, CT, F], BF16, tag="a16")
                nc.scalar.activation(
                    A16[:].rearrange("p c f -> p (c f)"),
                    X[:].rearrange("p c f -> p (c f)"),
                    Act.Relu, bias=relu_bias[:], scale=1.0,
                )
                staged[i].append(A16)

            def tail(i):
                X, Q16 = staged[i][0], staged[i][1]
                A16 = staged[i][4]
                X4 = X[:].rearrange("p (b i) f -> p b (i f)", b=NBt, i=blk)
                A4 = A16[:].rearrange("p (b i) f -> p b (i f)", b=NBt, i=blk)
                if tail_mode == "stt":
                    for b in range(NBt):
                        Qb = Q16[:, b].unsqueeze(1).broadcast_to(
                            [PD, blk, F])
                        nc.vector.scalar_tensor_tensor(
                            X4[:, b], A4[:, b], LIM, Qb,
                            op0=Alu.subtract, op1=Alu.mult,
                        )
                else:  # ttdirect: bf16 4x sub, then per-block bc TT
                    A2 = A16[:].rearrange("p c f -> p (c f)")
                    nc.vector.tensor_scalar(A2, A2, LIM, None,
                                            op0=Alu.subtract)
                    for b in range(NBt):
                        Qb = Q16[:, b].unsqueeze(1).broadcast_to(
                            [PD, blk, F])
                        nc.vector.tensor_tensor(X4[:, b], A4[:, b], Qb,
                                                op=Alu.mult)

            def post(i):
                ts_, cc_ = tl[i]
                X = staged.pop(i)[0]
                getattr(nc, store_engine).dma_start(orr[ts_, cc_], X[:])

            n = len(tl)
            for i in range(min(bufs, n)):
                issue_load(i)
            if mode == "copy":
                for i, (ts_, cc_) in enumerate(tl):
                    X = staged.pop(i)[0]
                    nc.sync.dma_start(orr[ts_, cc_], X[:])
                    if i + bufs < n:
                        issue_load(i + bufs)
            else:
                # skew: abs(i) | tree+mult(i-1) | round+relu(i-2) |
                #       stt(i-3) | store(i-4)
                for i in range(n):
                    h_abs(i)
                    if i >= 1:
                        h_tree(i - 1)
                        h_mult(i - 1)
                    if i >= 2:
                        mid(i - 2)
                    if i >= 3:
                        tail(i - 3)
                    if i >= 4:
                        post(i - 4)
                    j = i + bufs - 4
                    if bufs <= j < n:
                        issue_load(j)
                for j in (n - 1,):
                    h_tree(j)
                    h_mult(j)
                for j in (n - 2, n - 1):
                    if j >= 0:
                        mid(j)
                for j in (n - 3, n - 2, n - 1):
                    if j >= 0:
                        tail(j)
                for j in (n - 4, n - 3, n - 2, n - 1):
                    if j >= 0:
                        post(j)
    return nc


def _get_nc(mant: int, blk: int, arch: str = "v2", **kw):
    key = (mant, blk, arch, tuple(sorted(kw.items())))
    if key not in _BUILT:
        build = {"v1": _build, "v2": _build_v2}[arch]
        nc = build(mant, blk, **kw)
        if not nc.is_finalized():
            nc.finalize()
        _BUILT[key] = nc
    return _BUILT[key]


def kernel(activations, mantissa=7, blk=16, **_ignored):
    from concourse.bass_utils import run_bass_kernel_spmd

    mant = int(np.asarray(mantissa))
    blk = int(np.asarray(blk))
    x = np.asarray(activations, dtype=np.float32)
    assert x.shape == (N, C, D, H, W), x.shape
    assert blk == 16 and C % blk == 0

    nc = _get_nc(mant, blk)
    xf = x.reshape(N, C, S)
    in_maps = [{"x": np.ascontiguousarray(xf[n])} for n in range(N_CORES)]
    res = run_bass_kernel_spmd(nc, in_maps, list(range(N_CORES)))
    outs = [np.asarray(r["o"], dtype=np.float32) for r in res.results]
    return np.stack(outs, axis=0).reshape(N, C, D, H, W)

